# revision 64
# baseline (speedup 1.0000x reference)
"""Multi-head self-attention (B=8, S=1024, E=768, H=12, D=64) on 8 NeuronCores.

Sharding: data-parallel over batch - one batch element per core, weights
replicated, no collectives.  All matmuls run in bf16 (operands cast on
the way into SBUF); accumulation stays fp32 in PSUM.  Measured ~209us HW
exec (vs the 270us fp32r baseline), rel-rms error 5.5e-3.

Per-core dataflow:
  1. PE warmup burst (identity matmuls) trips the HAM clock-gate to 8/8
     within ~4us (the baseline ran its whole first half at 1.2GHz) while
     the first DMAs land.  All HBM loads ride ONE queue in priority order
     (x -> Wq/Wk interleaved -> Wv -> Wo): each DMA stripes over all 16
     engines, so concurrent queues only steal bandwidth from x.
  2. xT via PE transpose in bf16 (x cast on DVE first; bf16 transposes
     are 1 cyc/row and their LDWEIGHTS gets fast-weight-load).
  3. V scattered into V_ext[s, ktile, pair, parity, 128] = [V+bv | ones]
     (even head) / [ones | V+bv] (odd head), evacuated with 4 batched DVE
     adds per s-tile.  The ones-halves are written by ACT-copy
     (scale=0 bias=1) on the early-idle Scalar engine: a DVE/GpSimd
     memset SBUF-port-starved concurrent DVE casts ~11x.
  4. Attention runs as 12 "slots" (pair x q-half).  Each slot emits 8
     score groups; the two K=64 matmuls of a group carry explicit
     tile_position (0,0)/(64,0) and run CONCURRENT in the PE array
     (measured dt 4ns - this halves scores cost), into one [P,2,512]
     PSUM tile drained by one 1024-elem ACT exp (scale=1/8 folded in, no
     max subtraction: scores ~ N(0,1)).  Between groups the slot weaves
     "filler" PE work - the previous slot's attnV in 2-matmul chunks and
     the next pair's QK projection chains - so the PE never idles while
     the scores PSUM tiles recycle at ACT pace (bufs=2).
  5. attnV per head: one M=128 chain over 8 ktiles -> rows [attn|sums]
     (even) / [sums|attn] (odd).  Normalize: evacuate all four PSUM
     halves with aligned copies first (the att banks recycle in ~1.4us;
     the 3.3us DVE reciprocal runs off the PE critical path), swap the
     sums halves via SBUF->SBUF DMA on the gpsimd queue (on the sync
     queue a waiting swap head-of-line blocked the weight loads),
     reciprocal, one multiply into concatT (bf16).
  6. out = concatT.T @ Wo + bo per s-tile: st0/1 overlap the last slot,
     st2/3 the last ACT drain; the final pair's normalize is split into
     128-column chunks so each st4..7 chain starts as soon as its columns
     are normalized.

Notes on two hardware workarounds baked in here (from the baseline):
 - This walrus build rejects instructions carrying more than ~1-2 sync
   waits ("Too many sync wait commands"); _split_excess_waits and the
   patched TileContext tail hoist surplus waits onto standalone EVSEM ops.
   It also rejects custom-DVE ops ("ISA wrong length"), so no
   reciprocal_approx_fast.
 - DVE reads from PSUM with a partition-base offset different from the
   output's silently return wrong data, so every PSUM read here is
   partition-aligned (copies/muls on the 0:64 / 64:128 halves in place);
   only the SBUF->SBUF DMA swap crosses partition bases.
"""
import sys
sys.path.insert(0, "/opt/trn_rl_repo")
from contextlib import ExitStack

import numpy as np

import concourse.bass as bass
import concourse.tile as tile
from concourse import mybir
from concourse.bass_utils import run_bass_kernel_spmd
from concourse.masks import make_identity
from concourse.vector_clock import ScopedClock


def _split_drain_and_barrier(self, tick_clock, wait_clock):
    """TileContext tail with the final drain's waits split one-per-instruction."""
    drain_inst = self.nc.sync.drain()
    wait_clock.add_sem_waits(
        drain_inst.ins, ScopedClock({None: tick_clock.global_clock})
    )
    si = drain_inst.ins.sync_info
    waits = list(si.on_wait) if si is not None and si.on_wait else []
    if len(waits) > 1:
        si.on_wait = []
        by_num = {h.num: h for h in self.sems.allocated().values()}
        for w in waits:
            self.nc.sync.wait_ge(by_num[w.id], w.wait_value)
    self.nc.all_engine_barrier()
    popped = self.nc._tile_sem_poison_stack.pop()
    assert popped is self._sem_poison
    self.nc.clear_and_free_semaphores(list(self.sems.allocated().values()))
    self.nc.all_engine_barrier()


tile.TileContext._drain_and_barrier = _split_drain_and_barrier


def _split_excess_waits(nc):
    """Hoist excess per-instruction sync waits into standalone EVSEM waits."""
    counter = 0
    for f in nc.m.functions:
        for bb in f.blocks:
            insts = bb.instructions
            out = []
            for inst in insts:
                si = inst.sync_info
                cap = 2 if isinstance(inst, mybir.InstEventSemaphore) else 1
                if si is not None and si.on_wait and len(si.on_wait) > cap:
                    waits = list(si.on_wait)
                    for w in waits[cap:]:
                        counter += 1
                        ev = mybir.InstEventSemaphore(name=f"I-wsplit-{counter}")
                        ev.engine = inst.engine
                        ev.sync_info = mybir.SyncInfo(on_wait=[w], on_update=[])
                        out.append(ev)
                    si.on_wait = waits[:cap]
                out.append(inst)
            if len(out) != len(insts):
                insts[:] = out
    return counter

P = 128
S = 1024
E = 768
H = 12
D = 64
KT = E // P        # 6 e-tiles
ST = S // P        # 8 s-tiles
NPAIR = H // 2     # 6 head pairs
QTILE = 512
NQ = S // QTILE    # 2 q-tiles
ESLICES = [(0, 512), (512, 256)]

f32 = mybir.dt.float32
f32r = mybir.dt.float32r
bf16 = mybir.dt.bfloat16
EXP = mybir.ActivationFunctionType.Exp

_NC_CACHE = {}


def build(mm_dtype="bf16", e_dtype="bf16", warmup=40, recip="exact"):
    mdt = {"f32r": f32r, "f32": f32, "bf16": bf16}[mm_dtype]
    edt = {"f32r": f32r, "f32": f32, "bf16": bf16}[e_dtype]
    nc = bass.Bass()
    x_d = nc.declare_dram_parameter("x", [S, E], f32, isOutput=False)
    Wq_d = nc.declare_dram_parameter("Wq", [E, E], f32, isOutput=False)
    Wk_d = nc.declare_dram_parameter("Wk", [E, E], f32, isOutput=False)
    Wv_d = nc.declare_dram_parameter("Wv", [E, E], f32, isOutput=False)
    Wo_d = nc.declare_dram_parameter("Wo", [E, E], f32, isOutput=False)
    bq_d = nc.declare_dram_parameter("bq", [E], f32, isOutput=False)
    bk_d = nc.declare_dram_parameter("bk", [E], f32, isOutput=False)
    bv_d = nc.declare_dram_parameter("bv", [E], f32, isOutput=False)
    bo_d = nc.declare_dram_parameter("bo", [E], f32, isOutput=False)
    out_d = nc.declare_dram_parameter("out", [S, E], f32, isOutput=True)

    with ExitStack() as ctx:
        tc = ctx.enter_context(tile.TileContext(nc))
        singles = ctx.enter_context(tc.tile_pool(name="singles", bufs=1))
        xld = ctx.enter_context(tc.tile_pool(name="xld", bufs=3))
        xbp = ctx.enter_context(tc.tile_pool(name="xbp", bufs=2))
        wstg = ctx.enter_context(tc.tile_pool(name="wstg", bufs=6))
        wbig = ctx.enter_context(tc.tile_pool(name="wbig", bufs=2))
        qkp = ctx.enter_context(tc.tile_pool(name="qkp", bufs=2))
        ep = ctx.enter_context(tc.tile_pool(name="ep", bufs=2))
        np_pool = ctx.enter_context(tc.tile_pool(name="norm", bufs=2))
        outp = ctx.enter_context(tc.tile_pool(name="outp", bufs=2))
        bcast = ctx.enter_context(tc.tile_pool(name="bcast", bufs=2))
        # PSUM: S(2x2) + att(2x1) + mm(2x1) = 8 banks
        psS = ctx.enter_context(tc.tile_pool(name="psS", bufs=2, space="PSUM"))
        psA = ctx.enter_context(tc.tile_pool(name="psA", bufs=2, space="PSUM"))
        psM = ctx.enter_context(tc.tile_pool(name="psM", bufs=2, space="PSUM"))

        # ---- constants ----
        ident = singles.tile([P, P], mdt)
        make_identity(nc, ident)
        bq_sb = singles.tile([P, KT], f32)
        bk_sb = singles.tile([P, KT], f32)

        def bcast_load(dst, src_ap):  # [E] -> [P, E] partition-step-0 DMA
            nc.gpsimd.dma_start(
                out=dst,
                in_=bass.AP(tensor=src_ap.tensor, offset=src_ap.offset,
                            ap=[[0, P]] + [list(a) for a in src_ap.ap]))
        bv_bc = bcast.tile([P, E], f32, tag="bbc")
        bo_bc = bcast.tile([P, E], f32, tag="bbc")

        # ---- persistent big buffers ----
        xT = singles.tile([P, KT, S], mdt)              # x^T  [e_in, s]
        V_ext = singles.tile([P, ST, NPAIR, 2, P], edt)  # [s, kt, pair, par, .]
        concatT = singles.tile([P, NPAIR, S], mdt)       # attn^T by pair

        def v_ext_ones():
            """Fill V_ext's ones-halves on the (early-idle) Scalar engine.

            A DVE/GpSimd memset takes ~10us and SBUF-port-starves concurrent
            DVE casts (~11x slowdown measured); ACT-copy with scale=0 bias=1
            writes the same pattern off both engines.  The ones regions
            [pair, par0, 64:128] + [pair, par1, 0:64] are contiguous
            128-element runs at +64 within each 256-element pair block.
            """
            vb = V_ext[:]
            pstep = list(vb.ap[0])
            ib = ident[:]
            src = bass.AP(tensor=ib.tensor, offset=ib.offset,
                          ap=[list(ib.ap[0]), [0, NPAIR], [1, 2 * D]])
            for st in range(ST):
                dst = bass.AP(tensor=vb.tensor,
                              offset=vb.offset + st * (NPAIR * 2 * P) + D,
                              ap=[pstep, [2 * P, NPAIR], [1, 2 * D]])
                nc.scalar.activation(dst, src,
                                     mybir.ActivationFunctionType.Copy,
                                     bias=1.0, scale=0.0)

        # ---- PE warmup: trip the HAM clock-gate to 8/8 early and keep the
        # PE busy while the x DMAs land (no-dependency identity matmuls,
        # also woven between the transpose groups below).
        def warm_mms(n):
            for w in range(0, n, 4):
                pw = psM.tile([P, 512], f32, tag="mm", name="pwarm")
                for j in range(4):
                    nc.tensor.matmul(pw[:, j * P:(j + 1) * P], ident[:],
                                     ident[:], start=True, stop=True)

        warm_mms(warmup)
        # Pre-load the exp table set during warmup (first real exp otherwise
        # pays the ~2.7us ACT_TABLE_LOAD mid-attention).  Reads ident, not
        # a DMA'd tensor, so it never waits.
        actw = singles.tile([P, 4], f32)
        nc.scalar.activation(actw[:], ident[:, 0:4], EXP, scale=0.125)

        # ---- weight loading helpers ----
        # Each DMA stripes across all 16 engines, so full-size chunks are
        # fine; what matters is ONE queue in strict priority order (x first,
        # then Wq/Wk, Wv, Wo) - concurrent queues steal bandwidth from x.
        def stage_chunk(dst_slice, src_slice):
            stg = wstg.tile([P, E], f32, tag="ws", name="wstage")
            nc.sync.dma_start(stg[:], src_slice)
            nc.vector.tensor_copy(dst_slice, stg[:])

        def wload_big(dst_r, W_d):
            src = W_d[:].rearrange("(ko p) m -> p ko m", p=P)
            for j in range(KT):
                stage_chunk(dst_r[:, j, :], src[:, j, :])

        # ---- transpose x (bf16): x -> xT ----
        # x DMAs are emitted FIRST on the sync queue (they gate the whole
        # pipeline); weight loads follow them.
        for st in range(ST):
            x_sb = xld.tile([P, E], f32, tag="x")
            nc.sync.dma_start(x_sb[:], x_d[st * P:(st + 1) * P, :])
            xb = xbp.tile([P, E], mdt, tag="xb")
            nc.vector.tensor_copy(xb[:], x_sb[:])
            pt = psM.tile([P, KT, P], mdt, tag="mm", name="pt")
            for j in range(KT):
                nc.tensor.transpose(pt[:, j, :], xb[:, j * P:(j + 1) * P],
                                    ident[:])
            nc.vector.tensor_copy(xT[:, :, st * P:(st + 1) * P], pt[:])
            if st < ST - 1:
                warm_mms(8)  # fill the x-DMA arrival gaps, keep HAM warm

        # Bias loads AFTER the x tiles on their queues: they were delaying
        # the first x arrival (x0 gates the whole pipeline).
        nc.sync.dma_start(bq_sb[:], bq_d[:].rearrange("(o p) -> p o", p=P))
        nc.sync.dma_start(bk_sb[:], bk_d[:].rearrange("(o p) -> p o", p=P))
        bcast_load(bv_bc[:], bv_d[:])
        bcast_load(bo_bc[:], bo_d[:])

        # Full Wq/Wk into SBUF (bf16), interleaved ko-chunks so proj0's q
        # and k chains are DMA-step-paced together; Wv after; Wo last.
        wq_sb = singles.tile([P, KT, E], mdt)
        wk_sb = singles.tile([P, KT, E], mdt)
        Wq_re = Wq_d[:].rearrange("(ko p) m -> p ko m", p=P)
        Wk_re = Wk_d[:].rearrange("(ko p) m -> p ko m", p=P)
        for j in range(KT):
            stage_chunk(wq_sb[:, j, :], Wq_re[:, j, :])
            stage_chunk(wk_sb[:, j, :], Wk_re[:, j, :])
        Wv_sb = wbig.tile([P, KT, E], mdt, tag="wbig")
        wload_big(Wv_sb, Wv_d)
        v_ext_ones()  # on scalar: done by ~17us, attnV needs it by ~45us

        # ---- filler machinery: fine-grained PE work chunks ----
        # Each filler is a closure emitting ~2 matmuls (+ evacuation ops).
        fillers = []

        def emit_fillers(n):
            for _ in range(n):
                if fillers:
                    fillers.pop(0)()

        def vproj_st(st):
            """V projection for one s-tile -> V_ext (two PSUM slices)."""
            pv = psM.tile([P, 512], f32, tag="mm", name="pv")
            for k in range(KT):
                nc.tensor.matmul(pv[:], xT[:, k, st * P:(st + 1) * P],
                                 Wv_sb[:, k, 0:512],
                                 start=(k == 0), stop=(k == KT - 1))
            pvv = pv[:].rearrange("p (pr a d) -> p pr a d", a=2, d=D)
            bvv = bv_bc[:].rearrange("p (pr a d) -> p pr a d", a=2, d=D)
            nc.vector.tensor_add(V_ext[:, st, 0:4, 0, 0:D], pvv[:, :, 0, :],
                                 bvv[:, 0:4, 0, :])
            nc.vector.tensor_add(V_ext[:, st, 0:4, 1, D:P], pvv[:, :, 1, :],
                                 bvv[:, 0:4, 1, :])
            pv2 = psM.tile([P, 256], f32, tag="mm", name="pv2")
            for k in range(KT):
                nc.tensor.matmul(pv2[:], xT[:, k, st * P:(st + 1) * P],
                                 Wv_sb[:, k, 512:768],
                                 start=(k == 0), stop=(k == KT - 1))
            pvv2 = pv2[:].rearrange("p (pr a d) -> p pr a d", a=2, d=D)
            nc.vector.tensor_add(V_ext[:, st, 4:6, 0, 0:D], pvv2[:, :, 0, :],
                                 bvv[:, 4:6, 0, :])
            nc.vector.tensor_add(V_ext[:, st, 4:6, 1, D:P], pvv2[:, :, 1, :],
                                 bvv[:, 4:6, 1, :])

        qt_t, kt_t = {}, {}

        def proj_half(m, which, q2):
            """One 6-matmul chain: QT_m (or KT_m) for one q-half."""
            w = wq_sb if which == "q" else wk_sb
            bias = bq_sb if which == "q" else bk_sb
            if q2 == 0:
                t = qkp.tile([P, S], mdt, tag=which + "t", name=which + "t")
                (qt_t if which == "q" else kt_t)[m] = t
            else:
                t = (qt_t if which == "q" else kt_t)[m]
            qsl = slice(q2 * QTILE, (q2 + 1) * QTILE)
            pq = psM.tile([P, 512], f32, tag="mm", name="pq")
            for k in range(KT):
                nc.tensor.matmul(pq[:], w[:, k, m * P:(m + 1) * P],
                                 xT[:, k, qsl],
                                 start=(k == 0), stop=(k == KT - 1))
            nc.vector.tensor_scalar_add(t[:, qsl], pq[:], bias[:, m:m + 1])

        def attnv_chunks(m, q2, e, split_norm=1):
            """attnV as filler chunks: 8x (2 matmuls) + normalize (in
            `split_norm` column chunks; >1 only for the tail, where outproj
            consumes concatT 128 columns at a time).

            p_a rows: [attn_a | sums_a]; p_b rows: [sums_b | attn_b].
            Normalize reads the PSUM halves in place (partition-aligned),
            swaps only the reciprocals via SBUF->SBUF DMA.
            """
            qsl = slice(q2 * QTILE, (q2 + 1) * QTILE)
            st_ = {}

            def mk_kt(kt):
                def go():
                    if kt == 0:
                        st_["p_a"] = psA.tile([P, 512], f32, tag="att",
                                              name="p_a")
                        st_["p_b"] = psA.tile([P, 512], f32, tag="att",
                                              name="p_b")
                    nc.tensor.matmul(st_["p_a"][:], V_ext[:, kt, m, 0, :],
                                     e[:, kt, 0, :],
                                     start=(kt == 0), stop=(kt == ST - 1))
                    nc.tensor.matmul(st_["p_b"][:], V_ext[:, kt, m, 1, :],
                                     e[:, kt, 1, :],
                                     start=(kt == 0), stop=(kt == ST - 1))
                return go

            def mk_norm(c0, c1):
                def go():
                    # Evacuate the psum halves to SBUF first (aligned
                    # copies) so the att banks recycle immediately; the slow
                    # reciprocal (3.3us DVE) then runs off the PE critical
                    # path.  The swap rides the gpsimd DMA queue: a waiting
                    # swap on the sync queue head-of-line blocked the
                    # weight loads.
                    cs = slice(c0, c1)
                    qcs = slice(q2 * QTILE + c0, q2 * QTILE + c1)
                    p_a, p_b = st_["p_a"], st_["p_b"]
                    if c0 == 0:
                        st_["av"] = np_pool.tile([P, 512], edt, tag="av",
                                                 name="av")
                        st_["rt"] = np_pool.tile([P, 512], f32, tag="rt",
                                                 name="rt")
                        st_["rs"] = np_pool.tile([P, 512], f32, tag="rs",
                                                 name="rs")
                    av, rt, rs = st_["av"], st_["rt"], st_["rs"]
                    nc.vector.tensor_copy(av[0:D, cs], p_a[0:D, cs])
                    nc.vector.tensor_copy(av[D:P, cs], p_b[D:P, cs])
                    nc.vector.tensor_copy(rt[D:P, cs], p_a[D:P, cs])
                    nc.vector.tensor_copy(rt[0:D, cs], p_b[0:D, cs])
                    nc.gpsimd.dma_start(rs[0:D, cs], rt[D:P, cs])
                    nc.gpsimd.dma_start(rs[D:P, cs], rt[0:D, cs])
                    nc.vector.reciprocal(rs[:, cs], rs[:, cs])
                    nc.vector.tensor_mul(concatT[:, m, qcs], av[:, cs],
                                         rs[:, cs])
                return go

            w = 512 // split_norm
            return ([mk_kt(kt) for kt in range(ST)] +
                    [mk_norm(c * w, (c + 1) * w) for c in range(split_norm)])

        def outproj_st(st):
            o_sb = outp.tile([P, E], f32, tag="o")
            for noff, nsz in ESLICES:
                po = psM.tile([P, 512], f32, tag="mm", name="po")
                for k in range(KT):
                    nc.tensor.matmul(
                        po[:, :nsz],
                        concatT[:, k, st * P:(st + 1) * P],
                        Wo_sb[:, k, noff:noff + nsz],
                        start=(k == 0), stop=(k == KT - 1),
                    )
                nc.vector.tensor_add(o_sb[:, noff:noff + nsz], po[:, :nsz],
                                     bo_bc[:, noff:noff + nsz])
            nc.sync.dma_start(out_d[st * P:(st + 1) * P, :], o_sb[:])

        # ---- attention slot: 8 score groups, ACT-paced, fillers woven ----
        def slot(m, q2):
            qsl = slice(q2 * QTILE, (q2 + 1) * QTILE)
            e = ep.tile([P, ST, 2, QTILE], edt, tag="e", name="e")
            for g in range(ST):
                s = psS.tile([P, 2, 512], f32, tag="S", name="s")
                ksl = slice(g * P, (g + 1) * P)
                nc.tensor.matmul(s[:, 0, :], kt_t[m][0:D, ksl],
                                 qt_t[m][0:D, qsl], start=True, stop=True,
                                 tile_position=(0, 0))
                nc.tensor.matmul(s[:, 1, :], kt_t[m][D:P, ksl],
                                 qt_t[m][D:P, qsl], start=True, stop=True,
                                 tile_position=(64, 0))
                nc.scalar.activation(e[:, g, :, :], s[:], EXP, scale=0.125)
                emit_fillers(2)
            return e

        # ---- emission schedule ----
        # Early: pair-0 projections (first scores consumer), two V-proj
        # tiles dense; the rest of V-proj fills slot (0,0).
        for q2 in range(NQ):
            proj_half(0, "q", q2)
            warm_mms(4)
        for q2 in range(NQ):
            proj_half(0, "k", q2)
            warm_mms(4)
        vproj_st(0)
        warm_mms(4)
        vproj_st(1)
        warm_mms(4)

        # Wo: stage the whole [E,E] f32 in SBUF via chunked DMAs now (they
        # ride after the early weight loads on the sync queue), cast late in
        # one DVE op (data long arrived; no DVE head-of-line wait).
        Wo_sb = wbig.tile([P, KT, E], mdt, tag="wbig")
        Wo_stg = singles.tile([P, KT, E], f32)
        Wo_re = Wo_d[:].rearrange("(ko p) m -> p ko m", p=P)
        for j in range(KT):
            nc.sync.dma_start(Wo_stg[:, j, :], Wo_re[:, j, :])

        e_prev = {}
        for m in range(NPAIR):
            for q2 in range(NQ):
                # fillers for this slot: previous slot's attnV + next
                # pair's projection chains + leftover V-proj tiles.
                if (m, q2) == (0, 0):
                    # warm fillers woven in: the vproj chains pace on Wv
                    # chunk arrival and their sub-us stalls were enough to
                    # re-throttle the HAM clock-gate for ~14us.
                    for st in range(2, ST):
                        fillers.append(lambda st=st: vproj_st(st))
                        if st < 5:
                            fillers.append(lambda: warm_mms(4))
                else:
                    pm, pq2 = (m, 0) if q2 == 1 else (m - 1, 1)
                    sn = 4 if (pm, pq2) == (NPAIR - 1, 0) else 1
                    fillers.extend(attnv_chunks(pm, pq2,
                                                e_prev.pop((pm, pq2)),
                                                split_norm=sn))
                if m + 1 < NPAIR:
                    if q2 == 0:
                        fillers.append(lambda m=m: proj_half(m + 1, "q", 0))
                        fillers.append(lambda m=m: proj_half(m + 1, "q", 1))
                    else:
                        fillers.append(lambda m=m: proj_half(m + 1, "k", 0))
                        fillers.append(lambda m=m: proj_half(m + 1, "k", 1))
                if (m, q2) == (2, 0):
                    fillers.append(
                        lambda: nc.vector.tensor_copy(Wo_sb[:], Wo_stg[:]))
                if (m, q2) == (NPAIR - 1, 1):
                    # outproj st0/1 need only q2=0 concatT (normalized by
                    # this slot's attnv(5,0) filler) - start them here.
                    fillers.append(lambda: outproj_st(0))
                    fillers.append(lambda: outproj_st(1))
                e_prev[(m, q2)] = slot(m, q2)
                emit_fillers(len(fillers))

        # ---- tail: outproj for the q2=0 rows while the last slot's exps
        # finish, then the last attnV with its normalize split per 128
        # columns, each chunk immediately followed by the outproj s-tile
        # that consumes exactly those columns.
        tail = attnv_chunks(NPAIR - 1, 1, e_prev.pop((NPAIR - 1, 1)),
                            split_norm=4)
        outproj_st(2)
        for kt in range(4):
            tail[kt]()
        outproj_st(3)
        for kt in range(4, ST):
            tail[kt]()
        for c in range(4):
            tail[ST + c]()
            outproj_st(4 + c)

    _split_excess_waits(nc)
    return nc


def run_spmd(inputs, Wq, bq, Wk, bk, Wv, bv, Wo, bo,
             mm_dtype="bf16", e_dtype="bf16", trace=False):
    key = (mm_dtype, e_dtype)
    if key not in _NC_CACHE:
        _NC_CACHE[key] = build(mm_dtype, e_dtype)
    nc = _NC_CACHE[key]
    x = np.asarray(inputs, dtype=np.float32)
    common = {
        "Wq": np.asarray(Wq, np.float32), "Wk": np.asarray(Wk, np.float32),
        "Wv": np.asarray(Wv, np.float32), "Wo": np.asarray(Wo, np.float32),
        "bq": np.asarray(bq, np.float32), "bk": np.asarray(bk, np.float32),
        "bv": np.asarray(bv, np.float32), "bo": np.asarray(bo, np.float32),
    }
    in_maps = [dict(common, x=np.ascontiguousarray(x[b])) for b in range(x.shape[0])]
    res = run_bass_kernel_spmd(nc, in_maps, core_ids=list(range(len(in_maps))),
                               trace=trace)
    out = np.stack([res.results[b]["out"] for b in range(len(in_maps))], axis=0)
    return out, res


def kernel(inputs, Wq, bq, Wk, bk, Wv, bv, Wo, bo):
    out, _ = run_spmd(inputs, Wq, bq, Wk, bk, Wv, bv, Wo, bo)
    return out


# revision 67
# speedup vs baseline: 1.0299x; 1.0299x over previous
"""Multi-head self-attention (B=8, S=1024, E=768, H=12, D=64) on 8 NeuronCores.

Sharding: data-parallel over batch - one batch element per core, weights
replicated, no collectives.  All matmuls run in bf16 (operands cast on
the way into SBUF); accumulation stays fp32 in PSUM.  Measured ~209us HW
exec (vs the 270us fp32r baseline), rel-rms error 5.5e-3.

Per-core dataflow:
  1. PE warmup burst (identity matmuls) trips the HAM clock-gate to 8/8
     within ~4us (the baseline ran its whole first half at 1.2GHz) while
     the first DMAs land.  All HBM loads ride ONE queue in priority order
     (x -> Wq/Wk interleaved -> Wv -> Wo): each DMA stripes over all 16
     engines, so concurrent queues only steal bandwidth from x.
  2. xT via PE transpose in bf16 (x cast on DVE first; bf16 transposes
     are 1 cyc/row and their LDWEIGHTS gets fast-weight-load).
  3. V scattered into V_ext[s, ktile, pair, parity, 128] = [V+bv | ones]
     (even head) / [ones | V+bv] (odd head), evacuated with 4 batched DVE
     adds per s-tile.  The ones-halves are written by ACT-copy
     (scale=0 bias=1) on the early-idle Scalar engine: a DVE/GpSimd
     memset SBUF-port-starved concurrent DVE casts ~11x.
  4. Attention runs as 12 "slots" (pair x q-half).  Each slot emits 8
     score groups; the two K=64 matmuls of a group carry explicit
     tile_position (0,0)/(64,0) and run CONCURRENT in the PE array
     (measured dt 4ns - this halves scores cost), into one [P,2,512]
     PSUM tile drained by one 1024-elem ACT exp (scale=1/8 folded in, no
     max subtraction: scores ~ N(0,1)).  Between groups the slot weaves
     "filler" PE work - the previous slot's attnV in 2-matmul chunks and
     the next pair's QK projection chains - so the PE never idles while
     the scores PSUM tiles recycle at ACT pace (bufs=2).
  5. attnV per head: one M=128 chain over 8 ktiles -> rows [attn|sums]
     (even) / [sums|attn] (odd).  Normalize: evacuate all four PSUM
     halves with aligned copies first (the att banks recycle in ~1.4us;
     the 3.3us DVE reciprocal runs off the PE critical path), swap the
     sums halves via SBUF->SBUF DMA on the gpsimd queue (on the sync
     queue a waiting swap head-of-line blocked the weight loads),
     reciprocal, one multiply into concatT (bf16).
  6. out = concatT.T @ Wo + bo per s-tile: st0/1 overlap the last slot,
     st2/3 the last ACT drain; the final pair's normalize is split into
     128-column chunks so each st4..7 chain starts as soon as its columns
     are normalized.

Notes on two hardware workarounds baked in here (from the baseline):
 - This walrus build rejects instructions carrying more than ~1-2 sync
   waits ("Too many sync wait commands"); _split_excess_waits and the
   patched TileContext tail hoist surplus waits onto standalone EVSEM ops.
   It also rejects custom-DVE ops ("ISA wrong length"), so no
   reciprocal_approx_fast.
 - DVE reads from PSUM with a partition-base offset different from the
   output's silently return wrong data, so every PSUM read here is
   partition-aligned (copies/muls on the 0:64 / 64:128 halves in place);
   only the SBUF->SBUF DMA swap crosses partition bases.
"""
import sys
sys.path.insert(0, "/opt/trn_rl_repo")
from contextlib import ExitStack

import numpy as np

import concourse.bass as bass
import concourse.tile as tile
from concourse import mybir
from concourse.bass_utils import run_bass_kernel_spmd
from concourse.masks import make_identity
from concourse.vector_clock import ScopedClock


def _split_drain_and_barrier(self, tick_clock, wait_clock):
    """TileContext tail with the final drain's waits split one-per-instruction."""
    drain_inst = self.nc.sync.drain()
    wait_clock.add_sem_waits(
        drain_inst.ins, ScopedClock({None: tick_clock.global_clock})
    )
    si = drain_inst.ins.sync_info
    waits = list(si.on_wait) if si is not None and si.on_wait else []
    if len(waits) > 1:
        si.on_wait = []
        by_num = {h.num: h for h in self.sems.allocated().values()}
        for w in waits:
            self.nc.sync.wait_ge(by_num[w.id], w.wait_value)
    self.nc.all_engine_barrier()
    popped = self.nc._tile_sem_poison_stack.pop()
    assert popped is self._sem_poison
    self.nc.clear_and_free_semaphores(list(self.sems.allocated().values()))
    self.nc.all_engine_barrier()


tile.TileContext._drain_and_barrier = _split_drain_and_barrier


def _split_excess_waits(nc):
    """Hoist excess per-instruction sync waits into standalone EVSEM waits."""
    counter = 0
    for f in nc.m.functions:
        for bb in f.blocks:
            insts = bb.instructions
            out = []
            for inst in insts:
                si = inst.sync_info
                cap = 2 if isinstance(inst, mybir.InstEventSemaphore) else 1
                if si is not None and si.on_wait and len(si.on_wait) > cap:
                    waits = list(si.on_wait)
                    for w in waits[cap:]:
                        counter += 1
                        ev = mybir.InstEventSemaphore(name=f"I-wsplit-{counter}")
                        ev.engine = inst.engine
                        ev.sync_info = mybir.SyncInfo(on_wait=[w], on_update=[])
                        out.append(ev)
                    si.on_wait = waits[:cap]
                out.append(inst)
            if len(out) != len(insts):
                insts[:] = out
    return counter

P = 128
S = 1024
E = 768
H = 12
D = 64
KT = E // P        # 6 e-tiles
ST = S // P        # 8 s-tiles
NPAIR = H // 2     # 6 head pairs
QTILE = 512
NQ = S // QTILE    # 2 q-tiles
ESLICES = [(0, 512), (512, 256)]

f32 = mybir.dt.float32
f32r = mybir.dt.float32r
bf16 = mybir.dt.bfloat16
EXP = mybir.ActivationFunctionType.Exp

_NC_CACHE = {}


def build(mm_dtype="bf16", e_dtype="bf16", warmup=40, recip="exact"):
    mdt = {"f32r": f32r, "f32": f32, "bf16": bf16}[mm_dtype]
    edt = {"f32r": f32r, "f32": f32, "bf16": bf16}[e_dtype]
    nc = bass.Bass()
    x_d = nc.declare_dram_parameter("x", [S, E], f32, isOutput=False)
    Wq_d = nc.declare_dram_parameter("Wq", [E, E], f32, isOutput=False)
    Wk_d = nc.declare_dram_parameter("Wk", [E, E], f32, isOutput=False)
    Wv_d = nc.declare_dram_parameter("Wv", [E, E], f32, isOutput=False)
    Wo_d = nc.declare_dram_parameter("Wo", [E, E], f32, isOutput=False)
    bq_d = nc.declare_dram_parameter("bq", [E], f32, isOutput=False)
    bk_d = nc.declare_dram_parameter("bk", [E], f32, isOutput=False)
    bv_d = nc.declare_dram_parameter("bv", [E], f32, isOutput=False)
    bo_d = nc.declare_dram_parameter("bo", [E], f32, isOutput=False)
    out_d = nc.declare_dram_parameter("out", [S, E], f32, isOutput=True)

    with ExitStack() as ctx:
        tc = ctx.enter_context(tile.TileContext(nc))
        singles = ctx.enter_context(tc.tile_pool(name="singles", bufs=1))
        xld = ctx.enter_context(tc.tile_pool(name="xld", bufs=3))
        xbp = ctx.enter_context(tc.tile_pool(name="xbp", bufs=2))
        wstg = ctx.enter_context(tc.tile_pool(name="wstg", bufs=6))
        wbig = ctx.enter_context(tc.tile_pool(name="wbig", bufs=2))
        qkp = ctx.enter_context(tc.tile_pool(name="qkp", bufs=2))
        ep = ctx.enter_context(tc.tile_pool(name="ep", bufs=2))
        np_pool = ctx.enter_context(tc.tile_pool(name="norm", bufs=2))
        outp = ctx.enter_context(tc.tile_pool(name="outp", bufs=2))
        bcast = ctx.enter_context(tc.tile_pool(name="bcast", bufs=2))
        # PSUM: S(2x2) + att(2x1) + mm(2x1) = 8 banks
        psS = ctx.enter_context(tc.tile_pool(name="psS", bufs=2, space="PSUM"))
        psA = ctx.enter_context(tc.tile_pool(name="psA", bufs=2, space="PSUM"))
        psM = ctx.enter_context(tc.tile_pool(name="psM", bufs=2, space="PSUM"))

        # ---- constants ----
        ident = singles.tile([P, P], mdt)
        make_identity(nc, ident)
        bq_sb = singles.tile([P, KT], f32)
        bk_sb = singles.tile([P, KT], f32)

        def bcast_load(dst, src_ap):  # [E] -> [P, E] partition-step-0 DMA
            nc.gpsimd.dma_start(
                out=dst,
                in_=bass.AP(tensor=src_ap.tensor, offset=src_ap.offset,
                            ap=[[0, P]] + [list(a) for a in src_ap.ap]))
        bv_bc = bcast.tile([P, E], f32, tag="bbc")
        bo_bc = bcast.tile([P, E], f32, tag="bbc")

        # ---- persistent big buffers ----
        xT = singles.tile([P, KT, S], mdt)              # x^T  [e_in, s]
        V_ext = singles.tile([P, ST, NPAIR, 2, P], edt)  # [s, kt, pair, par, .]
        concatT = singles.tile([P, NPAIR, S], mdt)       # attn^T by pair

        def v_ext_ones():
            """Fill V_ext's ones-halves on the (early-idle) Scalar engine.

            A DVE/GpSimd memset takes ~10us and SBUF-port-starves concurrent
            DVE casts (~11x slowdown measured); ACT-copy with scale=0 bias=1
            writes the same pattern off both engines.  The ones regions
            [pair, par0, 64:128] + [pair, par1, 0:64] are contiguous
            128-element runs at +64 within each 256-element pair block.
            """
            vb = V_ext[:]
            pstep = list(vb.ap[0])
            ib = ident[:]
            src = bass.AP(tensor=ib.tensor, offset=ib.offset,
                          ap=[list(ib.ap[0]), [0, NPAIR], [1, 2 * D]])
            for st in range(ST):
                dst = bass.AP(tensor=vb.tensor,
                              offset=vb.offset + st * (NPAIR * 2 * P) + D,
                              ap=[pstep, [2 * P, NPAIR], [1, 2 * D]])
                nc.scalar.activation(dst, src,
                                     mybir.ActivationFunctionType.Copy,
                                     bias=1.0, scale=0.0)

        # ---- PE warmup: trip the HAM clock-gate to 8/8 early and keep the
        # PE busy while the x DMAs land (no-dependency identity matmuls,
        # also woven between the transpose groups below).
        def warm_mms(n):
            for w in range(0, n, 4):
                pw = psM.tile([P, 512], f32, tag="mm", name="pwarm")
                for j in range(4):
                    nc.tensor.matmul(pw[:, j * P:(j + 1) * P], ident[:],
                                     ident[:], start=True, stop=True)

        warm_mms(warmup)
        # Pre-load the exp table set during warmup (first real exp otherwise
        # pays the ~2.7us ACT_TABLE_LOAD mid-attention).  Reads ident, not
        # a DMA'd tensor, so it never waits.
        actw = singles.tile([P, 4], f32)
        nc.scalar.activation(actw[:], ident[:, 0:4], EXP, scale=0.125)

        # ---- weight loading helpers ----
        # Each DMA stripes across all 16 engines, so full-size chunks are
        # fine; what matters is ONE queue in strict priority order (x first,
        # then Wq/Wk, Wv, Wo) - concurrent queues steal bandwidth from x.
        def stage_chunk(dst_slice, src_slice):
            stg = wstg.tile([P, E], f32, tag="ws", name="wstage")
            nc.sync.dma_start(stg[:], src_slice)
            nc.vector.tensor_copy(dst_slice, stg[:])

        def wload_big(dst_r, W_d):
            src = W_d[:].rearrange("(ko p) m -> p ko m", p=P)
            for j in range(KT):
                stage_chunk(dst_r[:, j, :], src[:, j, :])

        # ---- transpose x (bf16): x -> xT ----
        # x DMAs are emitted FIRST on the sync queue (they gate the whole
        # pipeline); weight loads follow them.
        for st in range(ST):
            x_sb = xld.tile([P, E], f32, tag="x")
            nc.sync.dma_start(x_sb[:], x_d[st * P:(st + 1) * P, :])
            xb = xbp.tile([P, E], mdt, tag="xb")
            nc.vector.tensor_copy(xb[:], x_sb[:])
            pt = psM.tile([P, KT, P], mdt, tag="mm", name="pt")
            for j in range(KT):
                nc.tensor.transpose(pt[:, j, :], xb[:, j * P:(j + 1) * P],
                                    ident[:])
            nc.vector.tensor_copy(xT[:, :, st * P:(st + 1) * P], pt[:])
            if st < ST - 1:
                warm_mms(8)  # fill the x-DMA arrival gaps, keep HAM warm

        # Bias loads AFTER the x tiles on their queues: they were delaying
        # the first x arrival (x0 gates the whole pipeline).
        nc.sync.dma_start(bq_sb[:], bq_d[:].rearrange("(o p) -> p o", p=P))
        nc.sync.dma_start(bk_sb[:], bk_d[:].rearrange("(o p) -> p o", p=P))
        bcast_load(bv_bc[:], bv_d[:])
        bcast_load(bo_bc[:], bo_d[:])

        # Full Wq/Wk into SBUF (bf16), interleaved ko-chunks so proj0's q
        # and k chains are DMA-step-paced together; Wv after; Wo last.
        wq_sb = singles.tile([P, KT, E], mdt)
        wk_sb = singles.tile([P, KT, E], mdt)
        Wq_re = Wq_d[:].rearrange("(ko p) m -> p ko m", p=P)
        Wk_re = Wk_d[:].rearrange("(ko p) m -> p ko m", p=P)
        for j in range(KT):
            stage_chunk(wq_sb[:, j, :], Wq_re[:, j, :])
            stage_chunk(wk_sb[:, j, :], Wk_re[:, j, :])
        Wv_sb = wbig.tile([P, KT, E], mdt, tag="wbig")
        wload_big(Wv_sb, Wv_d)
        v_ext_ones()  # on scalar: done by ~17us, attnV needs it by ~45us

        # ---- filler machinery: fine-grained PE work chunks ----
        # Each filler is a closure emitting ~2 matmuls (+ evacuation ops).
        fillers = []

        def emit_fillers(n):
            for _ in range(n):
                if fillers:
                    fillers.pop(0)()

        def vproj_st(st):
            """V projection for one s-tile -> V_ext (two PSUM slices)."""
            pv = psM.tile([P, 512], f32, tag="mm", name="pv")
            for k in range(KT):
                nc.tensor.matmul(pv[:], xT[:, k, st * P:(st + 1) * P],
                                 Wv_sb[:, k, 0:512],
                                 start=(k == 0), stop=(k == KT - 1))
            pvv = pv[:].rearrange("p (pr a d) -> p pr a d", a=2, d=D)
            bvv = bv_bc[:].rearrange("p (pr a d) -> p pr a d", a=2, d=D)
            nc.vector.tensor_add(V_ext[:, st, 0:4, 0, 0:D], pvv[:, :, 0, :],
                                 bvv[:, 0:4, 0, :])
            nc.vector.tensor_add(V_ext[:, st, 0:4, 1, D:P], pvv[:, :, 1, :],
                                 bvv[:, 0:4, 1, :])
            pv2 = psM.tile([P, 256], f32, tag="mm", name="pv2")
            for k in range(KT):
                nc.tensor.matmul(pv2[:], xT[:, k, st * P:(st + 1) * P],
                                 Wv_sb[:, k, 512:768],
                                 start=(k == 0), stop=(k == KT - 1))
            pvv2 = pv2[:].rearrange("p (pr a d) -> p pr a d", a=2, d=D)
            nc.vector.tensor_add(V_ext[:, st, 4:6, 0, 0:D], pvv2[:, :, 0, :],
                                 bvv[:, 4:6, 0, :])
            nc.vector.tensor_add(V_ext[:, st, 4:6, 1, D:P], pvv2[:, :, 1, :],
                                 bvv[:, 4:6, 1, :])

        qt_t, kt_t = {}, {}

        def proj_half(m, which, q2):
            """One 6-matmul chain: QT_m (or KT_m) for one q-half."""
            w = wq_sb if which == "q" else wk_sb
            bias = bq_sb if which == "q" else bk_sb
            if q2 == 0:
                t = qkp.tile([P, S], mdt, tag=which + "t", name=which + "t")
                (qt_t if which == "q" else kt_t)[m] = t
            else:
                t = (qt_t if which == "q" else kt_t)[m]
            qsl = slice(q2 * QTILE, (q2 + 1) * QTILE)
            pq = psM.tile([P, 512], f32, tag="mm", name="pq")
            for k in range(KT):
                nc.tensor.matmul(pq[:], w[:, k, m * P:(m + 1) * P],
                                 xT[:, k, qsl],
                                 start=(k == 0), stop=(k == KT - 1))
            nc.vector.tensor_scalar_add(t[:, qsl], pq[:], bias[:, m:m + 1])

        def attnv_chunks(m, q2, e, split_norm=1):
            """attnV as filler chunks: 8x (2 matmuls) + normalize (in
            `split_norm` column chunks; >1 only for the tail, where outproj
            consumes concatT 128 columns at a time).

            p_a rows: [attn_a | sums_a]; p_b rows: [sums_b | attn_b].
            Normalize reads the PSUM halves in place (partition-aligned),
            swaps only the reciprocals via SBUF->SBUF DMA.
            """
            qsl = slice(q2 * QTILE, (q2 + 1) * QTILE)
            st_ = {}

            def mk_kt(kt):
                def go():
                    if kt == 0:
                        st_["p_a"] = psA.tile([P, 512], f32, tag="att",
                                              name="p_a")
                        st_["p_b"] = psA.tile([P, 512], f32, tag="att",
                                              name="p_b")
                    nc.tensor.matmul(st_["p_a"][:], V_ext[:, kt, m, 0, :],
                                     e[:, kt, 0, :],
                                     start=(kt == 0), stop=(kt == ST - 1))
                    nc.tensor.matmul(st_["p_b"][:], V_ext[:, kt, m, 1, :],
                                     e[:, kt, 1, :],
                                     start=(kt == 0), stop=(kt == ST - 1))
                return go

            def mk_norm(c0, c1):
                def go():
                    # Evacuate the psum halves to SBUF first (aligned
                    # copies) so the att banks recycle immediately; the slow
                    # reciprocal (3.3us DVE) then runs off the PE critical
                    # path.  The swap rides the gpsimd DMA queue: a waiting
                    # swap on the sync queue head-of-line blocked the
                    # weight loads.
                    cs = slice(c0, c1)
                    qcs = slice(q2 * QTILE + c0, q2 * QTILE + c1)
                    p_a, p_b = st_["p_a"], st_["p_b"]
                    if c0 == 0:
                        st_["av"] = np_pool.tile([P, 512], edt, tag="av",
                                                 name="av")
                        st_["rt"] = np_pool.tile([P, 512], f32, tag="rt",
                                                 name="rt")
                        st_["rs"] = np_pool.tile([P, 512], f32, tag="rs",
                                                 name="rs")
                    av, rt, rs = st_["av"], st_["rt"], st_["rs"]
                    nc.vector.tensor_copy(av[0:D, cs], p_a[0:D, cs])
                    nc.vector.tensor_copy(av[D:P, cs], p_b[D:P, cs])
                    nc.vector.tensor_copy(rt[D:P, cs], p_a[D:P, cs])
                    nc.vector.tensor_copy(rt[0:D, cs], p_b[0:D, cs])
                    nc.gpsimd.dma_start(rs[0:D, cs], rt[D:P, cs])
                    nc.gpsimd.dma_start(rs[D:P, cs], rt[0:D, cs])
                    nc.vector.reciprocal(rs[:, cs], rs[:, cs])
                    nc.vector.tensor_mul(concatT[:, m, qcs], av[:, cs],
                                         rs[:, cs])
                return go

            w = 512 // split_norm
            return ([mk_kt(kt) for kt in range(ST)] +
                    [mk_norm(c * w, (c + 1) * w) for c in range(split_norm)])

        def outproj_st(st):
            o_sb = outp.tile([P, E], f32, tag="o")
            for noff, nsz in ESLICES:
                po = psM.tile([P, 512], f32, tag="mm", name="po")
                for k in range(KT):
                    nc.tensor.matmul(
                        po[:, :nsz],
                        concatT[:, k, st * P:(st + 1) * P],
                        Wo_sb[:, k, noff:noff + nsz],
                        start=(k == 0), stop=(k == KT - 1),
                    )
                nc.vector.tensor_add(o_sb[:, noff:noff + nsz], po[:, :nsz],
                                     bo_bc[:, noff:noff + nsz])
            nc.sync.dma_start(out_d[st * P:(st + 1) * P, :], o_sb[:])

        # ---- attention slot: 8 score groups, ACT-paced, fillers woven ----
        def slot(m, q2):
            qsl = slice(q2 * QTILE, (q2 + 1) * QTILE)
            e = ep.tile([P, ST, 2, QTILE], edt, tag="e", name="e")
            for g in range(ST):
                s = psS.tile([P, 2, 512], f32, tag="S", name="s")
                ksl = slice(g * P, (g + 1) * P)
                nc.tensor.matmul(s[:, 0, :], kt_t[m][0:D, ksl],
                                 qt_t[m][0:D, qsl], start=True, stop=True,
                                 tile_position=(0, 0))
                nc.tensor.matmul(s[:, 1, :], kt_t[m][D:P, ksl],
                                 qt_t[m][D:P, qsl], start=True, stop=True,
                                 tile_position=(64, 0))
                nc.scalar.activation(e[:, g, :, :], s[:], EXP, scale=0.125)
                emit_fillers(2)
            return e

        # ---- emission schedule ----
        # Early: pair-0 projections (first scores consumer), two V-proj
        # tiles dense; the rest of V-proj fills slot (0,0).
        for q2 in range(NQ):
            proj_half(0, "q", q2)
            warm_mms(4)
        for q2 in range(NQ):
            proj_half(0, "k", q2)
            warm_mms(4)


        # Wo: stage the whole [E,E] f32 in SBUF via chunked DMAs now (they
        # ride after the early weight loads on the sync queue), cast late in
        # one DVE op (data long arrived; no DVE head-of-line wait).
        Wo_sb = wbig.tile([P, KT, E], mdt, tag="wbig")
        Wo_stg = singles.tile([P, KT, E], f32)
        Wo_re = Wo_d[:].rearrange("(ko p) m -> p ko m", p=P)
        for j in range(KT):
            nc.sync.dma_start(Wo_stg[:, j, :], Wo_re[:, j, :])

        e_prev = {}
        for m in range(NPAIR):
            for q2 in range(NQ):
                # fillers for this slot: previous slot's attnV + next
                # pair's projection chains + leftover V-proj tiles.
                if (m, q2) == (0, 0):
                    # Scores start right after proj0; the whole V projection
                    # runs as fillers of this slot.  The proj(1,q) chains go
                    # FIRST (no Wv dependency - a vproj chain popped before
                    # Wv lands would head-of-line block the score groups).
                    fillers.append(lambda: proj_half(1, "q", 0))
                    fillers.append(lambda: proj_half(1, "q", 1))
                    fillers.extend([lambda st=st: vproj_st(st)
                                    for st in range(ST)])
                else:
                    pm, pq2 = (m, 0) if q2 == 1 else (m - 1, 1)
                    sn = 4 if (pm, pq2) == (NPAIR - 1, 0) else 1
                    fillers.extend(attnv_chunks(pm, pq2,
                                                e_prev.pop((pm, pq2)),
                                                split_norm=sn))
                if m + 1 < NPAIR:
                    if q2 == 0:
                        if (m, q2) != (0, 0):  # (0,0) already queued its own
                            fillers.append(lambda m=m:
                                           proj_half(m + 1, "q", 0))
                            fillers.append(lambda m=m:
                                           proj_half(m + 1, "q", 1))
                    else:
                        fillers.append(lambda m=m: proj_half(m + 1, "k", 0))
                        fillers.append(lambda m=m: proj_half(m + 1, "k", 1))
                if (m, q2) == (2, 0):
                    fillers.append(
                        lambda: nc.vector.tensor_copy(Wo_sb[:], Wo_stg[:]))
                if (m, q2) == (NPAIR - 1, 1):
                    # outproj st0/1 need only q2=0 concatT (normalized by
                    # this slot's attnv(5,0) filler) - start them here.
                    fillers.append(lambda: outproj_st(0))
                    fillers.append(lambda: outproj_st(1))
                e_prev[(m, q2)] = slot(m, q2)
                emit_fillers(len(fillers))

        # ---- tail: outproj for the q2=0 rows while the last slot's exps
        # finish, then the last attnV with its normalize split per 128
        # columns, each chunk immediately followed by the outproj s-tile
        # that consumes exactly those columns.
        tail = attnv_chunks(NPAIR - 1, 1, e_prev.pop((NPAIR - 1, 1)),
                            split_norm=4)
        outproj_st(2)
        for kt in range(4):
            tail[kt]()
        outproj_st(3)
        for kt in range(4, ST):
            tail[kt]()
        for c in range(4):
            tail[ST + c]()
            outproj_st(4 + c)

    _split_excess_waits(nc)
    return nc


def run_spmd(inputs, Wq, bq, Wk, bk, Wv, bv, Wo, bo,
             mm_dtype="bf16", e_dtype="bf16", trace=False):
    key = (mm_dtype, e_dtype)
    if key not in _NC_CACHE:
        _NC_CACHE[key] = build(mm_dtype, e_dtype)
    nc = _NC_CACHE[key]
    x = np.asarray(inputs, dtype=np.float32)
    common = {
        "Wq": np.asarray(Wq, np.float32), "Wk": np.asarray(Wk, np.float32),
        "Wv": np.asarray(Wv, np.float32), "Wo": np.asarray(Wo, np.float32),
        "bq": np.asarray(bq, np.float32), "bk": np.asarray(bk, np.float32),
        "bv": np.asarray(bv, np.float32), "bo": np.asarray(bo, np.float32),
    }
    in_maps = [dict(common, x=np.ascontiguousarray(x[b])) for b in range(x.shape[0])]
    res = run_bass_kernel_spmd(nc, in_maps, core_ids=list(range(len(in_maps))),
                               trace=trace)
    out = np.stack([res.results[b]["out"] for b in range(len(in_maps))], axis=0)
    return out, res


def kernel(inputs, Wq, bq, Wk, bk, Wv, bv, Wo, bo):
    out, _ = run_spmd(inputs, Wq, bq, Wk, bk, Wv, bv, Wo, bo)
    return out
